# revision 33
# baseline (speedup 1.0000x reference)
"""Trainium2 Bass kernel: batched multi-head self-attention (B=16, N=1024, D=768, H=12).

Strategy
--------
Data-parallel over the batch: 16 batches / 8 NeuronCores = 2 batches per core.
Each core runs an identical (SPMD) Bass program over its shard.

Per-core math, all matmuls in bf16 with fp32 PSUM accumulation:
  * Host pre-transposes x to xT [D, T] (T = 2048 local tokens) and casts
    x / qkv_w / proj_w to bf16.  Every on-device matmul operand is then in
    its natural layout -- no on-device transposes:
      - Q^T,K^T [c, tok] = matmul(lhsT=qkv_w[:, c-tile], rhs=xT)
      - V [tok, c]       = matmul(lhsT=xT[:, tok-tile], rhs=qkv_w_v)
      - S^T [k, q]       = matmul(lhsT=K^T_h [hd, k-tile], rhs=Q^T_h [hd, q])
        (scores computed TRANSPOSED; no max-subtraction needed: |score|<~6)
      - exp on ScalarE straight out of PSUM, cast to bf16 into SBUF
      - out^T [hd, q]    = matmul(lhsT=[V_h | ones(64)], rhs=expT [k, q]);
        psum rows 64-127 = softmax denominator, replicated for free.
      - normalize: ScalarE copies the denominator rows out of PSUM, then
        DVE reciprocal_approx_fast -> tensor_mul
      - y [tok, e]       = matmul(lhsT=out^T [d, tok-tile], rhs=proj_w) + bias
  * Head pairs share the PE array for S^T: heads 2i/2i+1 live at partition
    bases 0/64 of Q^T/K^T, so their matmuls land on row-tiles (0,0)/(64,0)
    of the 64x128 PE configuration and execute concurrently.
  * ScalarE(exp) is the busiest engine (~220us/core), so the whole kernel
    is ONE global software pipeline: a minimal QKV prefix gets attention
    started ~6us in, then all 24 attention units (2 batches x 12) stream
    back-to-back on ScalarE while every other unit (remaining QKV of both
    batches, both output projections) is woven between them as PE filler,
    weighted by estimated PE time so the PE never starves.

kernel() takes full unsharded inputs, shards on host, runs all 8 cores via
run_bass_kernel_spmd, and re-assembles the full output.
"""

import numpy as np
import ml_dtypes

import concourse.bass as bass
import concourse.mybir as mybir
import concourse.tile as tile
from concourse import bacc
from concourse.bass_utils import run_bass_kernel_spmd

BF16 = mybir.dt.bfloat16
F32 = mybir.dt.float32

N_CORES = 8
B, SEQ, D = 16, 1024, 768
H, HD = 12, 64
BPC = B // N_CORES            # batches per core
T = BPC * SEQ                 # tokens per core
P = 128
KT = D // P                   # 6 contraction sub-tiles of 128
NQ = 512                      # moving free-dim per matmul (1 psum bank of fp32)
QT = SEQ // NQ                # 2 query tiles per batch
KTT = SEQ // P                # 8 key-token tiles per batch
NV = 384                      # V-projection output tile (2 per 768)
SCALE = HD ** -0.5
CTQ = 2 * D // P              # 12 channel tiles for Q|K


def _emit(tc, xT_d, wqkv_d, wproj_d, bias_d, y_d):
    nc = tc.nc
    from contextlib import ExitStack

    with ExitStack() as ctx:
        consts = ctx.enter_context(tc.tile_pool(name="consts", bufs=1))
        xt_pool = ctx.enter_context(tc.tile_pool(name="xt", bufs=2))
        qk_pool = ctx.enter_context(tc.tile_pool(name="qkT", bufs=2))
        v_pool = ctx.enter_context(tc.tile_pool(name="v", bufs=2))
        ot_pool = ctx.enter_context(tc.tile_pool(name="ot", bufs=2))
        e_pool = ctx.enter_context(tc.tile_pool(name="e", bufs=5))
        dn_pool = ctx.enter_context(tc.tile_pool(name="dn", bufs=2))
        rb_pool = ctx.enter_context(tc.tile_pool(name="rb", bufs=2))
        y_pool = ctx.enter_context(tc.tile_pool(name="y", bufs=2))
        mm_ps = ctx.enter_context(tc.tile_pool(name="mmps", bufs=2, space="PSUM"))
        st_ps = ctx.enter_context(tc.tile_pool(name="stps", bufs=2, space="PSUM"))
        av_ps = ctx.enter_context(tc.tile_pool(name="avps", bufs=2, space="PSUM"))

        # ---- PE warmup ----
        # The HAM clock gate keeps the PE at 1.2 GHz until it has been busy
        # for a full ~3.4us window.  A dozen junk matmuls on a memset tile
        # flip it to 2.4 GHz while the first input DMAs are still in flight,
        # so the real QKV matmuls start warm instead of paying the half-rate
        # penalty for their first ~16us.
        warm_sb = consts.tile([P, P], BF16, name="warm")
        nc.gpsimd.memset(warm_sb, 0.5)
        wps = st_ps.tile([P, 2, NQ], F32, tag="st", name="warmps")

        def junk_mms(n):
            for i in range(n):
                nc.tensor.matmul(
                    wps[:, i % 2, 0:P],
                    lhsT=warm_sb,
                    rhs=warm_sb,
                    start=True,
                    stop=True,
                )

        junk_mms(36)

        # ---- constants ----
        # Column-block-ordered loads: the first three descriptors carry
        # exactly what the prefix units (Q ct0, K ct6, xT q-half 0) need, so
        # real matmuls start ~4us in instead of ~10us.
        xT_full = xT_d[:].rearrange("(po pi) t -> pi po t", pi=P)  # [128, 6, T]
        wqkv_full = wqkv_d[:].rearrange("(po pi) c -> pi po c", pi=P)

        wqkv_sb = consts.tile([P, KT, 3 * D], BF16)
        xT0_sb = xt_pool.tile([P, KT, SEQ], BF16, tag="xt", name="xT0")
        nc.sync.dma_start(out=wqkv_sb[:, :, 0:P], in_=wqkv_full[:, :, 0:P])
        # The first Q chain is gated per-descriptor, so ship its xT half as
        # per-kt chunks: matmul kt can start as soon as slice kt lands.
        for kt in range(KT):
            nc.sync.dma_start(
                out=xT0_sb[:, kt, 0:NQ], in_=xT_full[:, kt, 0:NQ]
            )
        nc.sync.dma_start(out=wqkv_sb[:, :, D : D + P], in_=wqkv_full[:, :, D : D + P])
        # V columns (first half), then the rest in rough consumption order.
        nc.sync.dma_start(
            out=wqkv_sb[:, :, 2 * D : 2 * D + NV],
            in_=wqkv_full[:, :, 2 * D : 2 * D + NV],
        )
        nc.sync.dma_start(out=xT0_sb[:, :, NQ:SEQ], in_=xT_full[:, :, NQ:SEQ])
        nc.sync.dma_start(out=wqkv_sb[:, :, P:D], in_=wqkv_full[:, :, P:D])
        nc.sync.dma_start(
            out=wqkv_sb[:, :, D + P : 2 * D], in_=wqkv_full[:, :, D + P : 2 * D]
        )
        nc.sync.dma_start(
            out=wqkv_sb[:, :, 2 * D + NV :], in_=wqkv_full[:, :, 2 * D + NV :]
        )
        wproj_sb = consts.tile([P, KT, D], BF16)
        nc.sync.dma_start(
            out=wproj_sb, in_=wproj_d[:].rearrange("(po pi) c -> pi po c", pi=P)
        )
        bias_sb = consts.tile([P, D], F32)
        b_ap = bias_d[:]
        bias_bcast = bass.AP(
            tensor=b_ap.tensor, offset=b_ap.offset, ap=[[0, P], *b_ap.ap]
        )
        nc.sync.dma_start(out=bias_sb, in_=bias_bcast)

        # per-batch tiles, allocated lazily
        state = {}

        def batch_state(b):
            if b in state:
                return state[b]
            if b == 0:
                xT_sb = xT0_sb
            else:
                xT_sb = xt_pool.tile([P, KT, SEQ], BF16, tag="xt", name=f"xT{b}")
                nc.sync.dma_start(
                    out=xT_sb, in_=xT_full[:, :, b * SEQ : (b + 1) * SEQ]
                )
            qkT_sb = qk_pool.tile([P, CTQ, SEQ], BF16, tag="qkT", name=f"qkT{b}")
            v_sb = v_pool.tile([P, KTT, H, 2 * HD], BF16, tag="v", name=f"v{b}")
            nc.gpsimd.memset(v_sb[:, :, :, HD : 2 * HD], 1.0)
            outT_sb = ot_pool.tile([P, KT, SEQ], BF16, tag="ot", name=f"ot{b}")
            state[b] = (xT_sb, qkT_sb, v_sb, outT_sb)
            return state[b]

        # ---------- unit constructors ----------
        # Each unit is (fn, est_pe_time_ns).

        def qk_unit(b, ct, qt):
            """One Q^T/K^T channel-tile x query-half: 6 matmuls + copy."""
            def u():
                xT_sb, qkT_sb, _, _ = batch_state(b)
                ps = mm_ps.tile([P, NQ], F32, tag="mm", name=f"qk{b}_{ct}_{qt}")
                for kt in range(KT):
                    nc.tensor.matmul(
                        ps,
                        lhsT=wqkv_sb[:, kt, ct * P : (ct + 1) * P],
                        rhs=xT_sb[:, kt, qt * NQ : (qt + 1) * NQ],
                        start=(kt == 0),
                        stop=(kt == KT - 1),
                    )
                nc.vector.tensor_copy(
                    out=qkT_sb[:, ct, qt * NQ : (qt + 1) * NQ], in_=ps
                )
            return (u, 1300)

        def v_unit(b, tt, nt):
            """One V token-tile x half-channel: 6 matmuls + copy."""
            def u():
                xT_sb, _, v_sb, _ = batch_state(b)
                ps = mm_ps.tile([P, NQ], F32, tag="mm", name=f"v{b}_{tt}_{nt}")
                for kt in range(KT):
                    nc.tensor.matmul(
                        ps[:, :NV],
                        lhsT=xT_sb[:, kt, tt * P : (tt + 1) * P],
                        rhs=wqkv_sb[
                            :, kt, 2 * D + nt * NV : 2 * D + (nt + 1) * NV
                        ],
                        start=(kt == 0),
                        stop=(kt == KT - 1),
                    )
                nc.vector.tensor_copy(
                    out=v_sb[:, tt, nt * 6 : (nt + 1) * 6, 0:HD],
                    in_=ps[:, :NV].rearrange("p (h d) -> p h d", d=HD),
                )
            return (u, 1000)

        def attn_unit(b, hp, qt):
            """One (head-pair, q-tile): 8 S^T pairs + 8 exps + 16 AVs + norm."""
            def u():
                _, qkT_sb, v_sb, outT_sb = batch_state(b)
                avs = [
                    av_ps.tile([P, NQ], F32, tag="av", name=f"av{b}_{hp}_{qt}_{i}")
                    for i in range(2)
                ]
                epairs = []

                def st_exp(kt):
                    # both heads' S^T into one 2-bank psum tile so the
                    # row-tiled pair issues back-to-back, then one wide
                    # exp covers both banks.
                    stp = st_ps.tile(
                        [P, 2, NQ], F32, tag="st", name=f"st{b}_{hp}_{qt}_{kt}"
                    )
                    for hi in range(2):
                        base = hi * HD
                        nc.tensor.matmul(
                            stp[:, hi, :],
                            lhsT=qkT_sb[
                                base : base + HD, KT + hp, kt * P : (kt + 1) * P
                            ],
                            rhs=qkT_sb[
                                base : base + HD, hp, qt * NQ : (qt + 1) * NQ
                            ],
                            start=True,
                            stop=True,
                        )
                    e_t = e_pool.tile(
                        [P, 2, NQ], BF16, tag="e", name=f"e{b}_{hp}_{qt}_{kt}"
                    )
                    nc.scalar.activation(
                        out=e_t,
                        in_=stp,
                        func=mybir.ActivationFunctionType.Exp,
                        scale=SCALE,
                    )
                    epairs.append(e_t)

                def av(hi, kt):
                    nc.tensor.matmul(
                        avs[hi],
                        lhsT=v_sb[:, kt, 2 * hp + hi, :],
                        rhs=epairs[kt][:, hi, :],
                        start=(kt == 0),
                        stop=(kt == KTT - 1),
                        skip_group_check=True,
                    )

                def normalize(hi):
                    base = hi * HD
                    den = dn_pool.tile(
                        [HD, NQ], F32, tag="den", name=f"den{b}_{hp}_{qt}_{hi}"
                    )
                    # ScalarE does the denominator evacuation: the in-order
                    # DVE queue is what the next filler's PSUM-freeing CAST
                    # waits behind, so keep normalize's DVE share minimal.
                    nc.scalar.copy(out=den, in_=avs[hi][HD : 2 * HD, :])
                    rb = rb_pool.tile(
                        [HD, NQ], F32, tag="rb", name=f"rb{b}_{hp}_{qt}_{hi}"
                    )
                    nc.vector.reciprocal_approx_fast(out=rb, in_=den)
                    nc.vector.tensor_mul(
                        out=outT_sb[
                            base : base + HD, hp, qt * NQ : (qt + 1) * NQ
                        ],
                        in0=avs[hi][0:HD, :],
                        in1=rb,
                    )

                # depth-2 software pipeline: exp(kt) is consumed two
                # steps later, so the tail AVs never wait on ScalarE.
                for kt in range(KTT):
                    st_exp(kt)
                    if kt >= 2:
                        av(0, kt - 2)
                        av(1, kt - 2)
                for kt in (KTT - 2, KTT - 1):
                    av(0, kt)
                    av(1, kt)
                normalize(0)
                normalize(1)
            return (u, 5200)

        def proj_unit(b, tt):
            """One token-tile of the output projection: 12 matmuls + bias + DMA."""
            def u():
                _, _, _, outT_sb = batch_state(b)
                y_sb = y_pool.tile([P, D], F32, tag="y", name=f"y{b}_{tt}")
                for n0, nw in ((0, NQ), (NQ, D - NQ)):
                    ps = mm_ps.tile([P, NQ], F32, tag="mm", name=f"p{b}_{tt}_{n0}")
                    for dt2 in range(KT):
                        nc.tensor.matmul(
                            ps[:, :nw],
                            lhsT=outT_sb[:, dt2, tt * P : (tt + 1) * P],
                            rhs=wproj_sb[:, dt2, n0 : n0 + nw],
                            start=(dt2 == 0),
                            stop=(dt2 == KT - 1),
                        )
                    nc.vector.tensor_add(
                        out=y_sb[:, n0 : n0 + nw],
                        in0=ps[:, :nw],
                        in1=bias_sb[:, n0 : n0 + nw],
                    )
                nc.sync.dma_start(
                    out=y_d[b * SEQ + tt * P : b * SEQ + (tt + 1) * P, :],
                    in_=y_sb,
                )
            return (u, 1950)

        # ---------- emission schedule: one global pipeline ----------
        #
        # Tile derives dependencies from EMISSION order, so every producer
        # must be emitted before its consumer.  attn(b, hp, qt) needs:
        #   Q(b, ct=hp, qt);  K(b, ct=6+hp, both q-halves);
        #   V(b, tt=0..7, nt=hp//3).
        # The filler queue is ordered roughly in consumption order; before
        # each attn unit all of its outstanding producers are force-emitted,
        # and beyond that fillers are woven in proportionally (by estimated
        # PE time) to keep the PE fed through the ScalarE-heavy stretches.

        def qkv_fillers(b):
            fs = []
            fs.append((("q", b, 0, 0), qk_unit(b, 0, 0)))
            fs.append((("q", b, KT, 0), qk_unit(b, KT, 0)))
            fs.append((("v", b, 0, 0), v_unit(b, 0, 0)))
            fs.append((("v", b, 1, 0), v_unit(b, 1, 0)))
            fs.append((("q", b, KT, 1), qk_unit(b, KT, 1)))
            fs.append((("q", b, 0, 1), qk_unit(b, 0, 1)))
            for tt in range(2, KTT):
                fs.append((("v", b, tt, 0), v_unit(b, tt, 0)))
            for hp in (1, 2):
                fs.append((("q", b, hp, 0), qk_unit(b, hp, 0)))
                fs.append((("q", b, KT + hp, 0), qk_unit(b, KT + hp, 0)))
                fs.append((("q", b, KT + hp, 1), qk_unit(b, KT + hp, 1)))
                fs.append((("q", b, hp, 1), qk_unit(b, hp, 1)))
            for tt in range(0, 4):
                fs.append((("v", b, tt, 1), v_unit(b, tt, 1)))
            for hp in (3,):
                fs.append((("q", b, hp, 0), qk_unit(b, hp, 0)))
                fs.append((("q", b, KT + hp, 0), qk_unit(b, KT + hp, 0)))
                fs.append((("q", b, KT + hp, 1), qk_unit(b, KT + hp, 1)))
                fs.append((("q", b, hp, 1), qk_unit(b, hp, 1)))
            for tt in range(4, KTT):
                fs.append((("v", b, tt, 1), v_unit(b, tt, 1)))
            for hp in (4, 5):
                fs.append((("q", b, hp, 0), qk_unit(b, hp, 0)))
                fs.append((("q", b, KT + hp, 0), qk_unit(b, KT + hp, 0)))
                fs.append((("q", b, KT + hp, 1), qk_unit(b, KT + hp, 1)))
                fs.append((("q", b, hp, 1), qk_unit(b, hp, 1)))
            return fs

        batch_state(0)
        batch_state(1)  # start xT(b1) DMA behind the early compute

        def attn_deps(b, hp, qt):
            deps = [("q", b, hp, qt), ("q", b, KT + hp, 0), ("q", b, KT + hp, 1)]
            deps += [("v", b, tt, hp // 3) for tt in range(KTT)]
            return deps

        attn_stream = [
            (attn_unit(b, hp, qt), attn_deps(b, hp, qt))
            for b in range(BPC)
            for qt in range(QT)
            for hp in range(H // 2)
        ]
        # Phase A: the whole of batch-0 QKV runs first (ScalarE idles, but
        # the engine-overlap stays in the baseline's envelope); attention +
        # batch-1 QKV + projections then stream as one weave.
        for n, (_, (u, _)) in enumerate(qkv_fillers(0)):
            u()
            if n < 8:
                # Keep the HAM activity window busy while the early units
                # run DMA-paced, without a long junk-only prefix blocking
                # ready real matmuls behind it in the priority queue.
                junk_mms(3)
        fillq = qkv_fillers(1)
        # proj(b, tt) needs attn(b, qt=tt//4, all hp): append to the queue
        # once the corresponding attn units have been emitted.  proj(b0, qt1)
        # is held back as the endgame reserve: the last attention units have
        # no other PE filler left, and the resulting >3.4us PE-idle windows
        # re-throttle the HAM clock gate for the whole projection tail.
        proj_avail = {
            (b * QT + qt + 1) * (H // 2): [
                (("p", b, tt), proj_unit(b, tt)) for tt in range(qt * 4, qt * 4 + 4)
            ]
            for b in range(BPC)
            for qt in range(QT)
            if not (b == 0 and qt == 1)
        }
        endgame_reserve = {
            20 + i: (("p", 0, 4 + i), proj_unit(0, 4 + i)) for i in range(4)
        }

        total_attn = sum(c for (_, c), _ in attn_stream)
        total_fill = sum(c for _, (_, c) in fillq) + sum(
            c for us in proj_avail.values() for _, (_, c) in us
        )
        pos = {}
        emitted = []
        fill_done = 0

        def grow_queue(items):
            for key, uc in items:
                pos[key] = len(emitted)
                fillq.append((key, uc))
                emitted.append(False)

        emitted = [False] * len(fillq)
        pos = {key: i for i, (key, _) in enumerate(fillq)}

        def emit_filler(j):
            nonlocal fill_done
            if not emitted[j]:
                emitted[j] = True
                fu, fc = fillq[j][1]
                fu()
                fill_done += fc

        attn_done = 0
        scan = 0  # next queue index for proportional weave
        for i, ((u, c), deps) in enumerate(attn_stream):
            if i in proj_avail:
                grow_queue(proj_avail.pop(i))
            for dkey in deps:
                if dkey in pos:
                    emit_filler(pos[dkey])
            u()
            if i in endgame_reserve:
                endgame_reserve.pop(i)[1][0]()
            attn_done += c
            want = total_fill * attn_done // total_attn
            while scan < len(fillq) and fill_done < want:
                emit_filler(scan)
                scan += 1
        for key in sorted(proj_avail):
            grow_queue(proj_avail.pop(key))
        # The projection tail runs while ScalarE/DVE drain the last attention
        # unit; short junk-matmul bursts between units keep the PE's HAM
        # activity window busy so the tail doesn't run at half clock.
        jps = st_ps.tile([P, 2, NQ], F32, tag="st", name="tailwarm")
        for j in range(len(fillq)):
            if not emitted[j]:
                for i in range(8):
                    nc.tensor.matmul(
                        jps[:, i % 2, 0:P],
                        lhsT=warm_sb,
                        rhs=warm_sb,
                        start=True,
                        stop=True,
                    )
            emit_filler(j)


def _build_program():
    nc = bacc.Bacc()
    xT_d = nc.declare_dram_parameter("xT", [D, T], BF16, isOutput=False)
    wqkv_d = nc.declare_dram_parameter("wqkv", [D, 3 * D], BF16, isOutput=False)
    wproj_d = nc.declare_dram_parameter("wproj", [D, D], BF16, isOutput=False)
    bias_d = nc.declare_dram_parameter("bias", [D], F32, isOutput=False)
    y_d = nc.declare_dram_parameter("y", [T, D], F32, isOutput=True)

    with tile.TileContext(nc) as tc:
        _emit(tc, xT_d, wqkv_d, wproj_d, bias_d, y_d)
    nc.compile()
    return nc


_NC = None


def _get_nc():
    global _NC
    if _NC is None:
        _NC = _build_program()
    return _NC


def _prep_in_maps(x, qkv_w, proj_w, proj_b):
    bf16 = ml_dtypes.bfloat16
    wq = np.ascontiguousarray(np.asarray(qkv_w).astype(bf16))
    wp = np.ascontiguousarray(np.asarray(proj_w).astype(bf16))
    pb = np.ascontiguousarray(np.asarray(proj_b).astype(np.float32))
    x = np.asarray(x)
    in_maps = []
    for c in range(N_CORES):
        xc = x[c * BPC : (c + 1) * BPC].reshape(T, D).astype(bf16)
        xTc = np.ascontiguousarray(xc.T)  # [D, T] bf16
        in_maps.append({"xT": xTc, "wqkv": wq, "wproj": wp, "bias": pb})
    return in_maps


def _run(x, qkv_w, proj_w, proj_b, **spmd_kwargs):
    nc = _get_nc()
    in_maps = _prep_in_maps(x, qkv_w, proj_w, proj_b)
    res = run_bass_kernel_spmd(nc, in_maps, core_ids=list(range(N_CORES)), **spmd_kwargs)
    y = np.stack([res.results[c]["y"] for c in range(N_CORES)])  # [8, T, D]
    return y.reshape(B, SEQ, D).astype(np.float32), res


def kernel(x, qkv_w, proj_w, proj_b):
    y, _ = _run(x, qkv_w, proj_w, proj_b)
    return y
